# revision 7
# baseline (speedup 1.0000x reference)
"""Hard-triplet miner for Trainium2, 8-core SPMD.

Per core: a [1024, 8192] strip of w = G - 2*[same_label] where
G = x_norm @ x_norm.T.  Since sqrt/shifts are monotone: hardest negative =
argmax_j w, hardest positive = argmin_j w.

Steady-state structure per 128-row tile (8 per core):
  ScalarE : nesc = |2*lab - 2*lab_i|, eqsc = relu(2 - nesc)   (2 passes)
  PE      : 16 fp32r matmuls (1 cycle/row -> 4x faster than fp32)
  DVE     : exactly 3 full passes over [128, 8192]:
              1. 8x tensor_tensor_reduce: w_ct = psum_ct - eqsc_ct with a
                 chained running MIN accumulator (min lands in inmaxs slot 1)
              2. tensor_reduce MAX over w       (lands in inmaxs slot 0)
              3. max_index over w with in_max = [max, min, PAD x6] ->
                 writes [argmax, argmin, ...] straight into the idx staging
                 dump (no per-tile copies)
  keep = (max > -0.9) & (min < -0.9), computed once per iteration on
  strided views of the staging buffers; outputs decoded host-side from a
  single [128, 64] dump.
"""

import numpy as np

import concourse.bacc as bacc
import concourse.bass as bass
import concourse.mybir as mybir
import concourse.tile as tile
from concourse import masks
from concourse.bass_utils import run_bass_kernel_spmd

F32 = mybir.dt.float32
F32R = mybir.dt.float32r
BF16 = mybir.dt.bfloat16
U32 = mybir.dt.uint32

N = 8192          # total rows
D = 128           # embed dim
NCORES = 8
STRIP = N // NCORES       # 1024 anchor rows per core
RT = STRIP // 128         # 8 row-tiles per core
CT_W = 1024               # column-tile width for psum/ttr
PAD_VAL = 3.0e38
MIN_INIT = 3.0e38


def build_program(k_repeat: int = 1, use_for_i: bool = False, n: int = N,
                  strip: int = STRIP, debug_level: int = 0,
                  ct_w: int = CT_W, mask_f32: bool = False):
    """Build the SPMD program (identical on all cores).  n/strip/ct_w
    shrinkable for simulator validation."""
    rt_n = strip // 128
    ct_w = min(ct_w, n)
    ct_n = n // ct_w
    t_full = n // 128

    nc = bacc.Bacc("TRN2", target_bir_lowering=False, debug=False,
                   num_devices=NCORES)

    x_full = nc.dram_tensor("x_full", [n, D], F32, kind="ExternalInput")
    x_strip = nc.dram_tensor("x_strip", [strip, D], F32, kind="ExternalInput")
    lab_full = nc.dram_tensor("lab_full", [1, n], F32, kind="ExternalInput")
    lab_strip = nc.dram_tensor("lab_strip", [128, rt_n], F32,
                               kind="ExternalInput")
    idx_out = nc.dram_tensor("idx_out", [128, 8 * rt_n], U32,
                             kind="ExternalOutput")
    keep_out = nc.dram_tensor("keep_out", [128, rt_n], F32,
                              kind="ExternalOutput")

    with tile.TileContext(nc) as tc:
        with (
            tc.tile_pool(name="persist", bufs=1) as persist,
            tc.tile_pool(name="rowp", bufs=3) as rowp,
            tc.tile_pool(name="maskp", bufs=2) as maskp,
            tc.tile_pool(name="nescp", bufs=1) as nescp,
            tc.tile_pool(name="wp", bufs=1) as wp,
            tc.tile_pool(name="smalls", bufs=4) as smalls,
            tc.tile_pool(name="psum_pro", bufs=2,
                         space=bass.MemorySpace.PSUM) as psum_pro,
            tc.tile_pool(name="psum_main", bufs=3,
                         space=bass.MemorySpace.PSUM) as psum_main,
        ):
            ident = persist.tile([128, 128], F32)
            masks.make_identity(nc, ident[:])

            xT = persist.tile([128, n], F32R, tag="xT")
            xsT = persist.tile([128, strip], F32R, tag="xsT")
            labrep = persist.tile([128, n], BF16, tag="labrep")
            labsT = persist.tile([128, rt_n], F32, tag="labsT")
            ones1 = persist.tile([1, 128], F32, tag="ones1")
            nc.gpsimd.memset(ones1[:], 1.0)
            lab1 = persist.tile([1, n], F32, tag="lab1")

            nc.sync.dma_start(lab1[:], lab_full[:])
            nc.sync.dma_start(labsT[:], lab_strip[:])

            bias2 = persist.tile([128, 1], F32, tag="bias2")
            nc.gpsimd.memset(bias2[:], 2.0)
            bias0 = persist.tile([128, 1], F32, tag="bias0")
            nc.gpsimd.memset(bias0[:], 0.0)

            # --- normalize + transpose: build xT (all rows) and xsT (strip) ---
            def norm_transpose(dst, src_dram, tiles):
                for t in range(tiles):
                    row = rowp.tile([128, D], F32, tag="row")
                    nc.sync.dma_start(row[:], src_dram[t * 128:(t + 1) * 128, :])
                    sq = rowp.tile([128, D], F32, tag="sq")
                    ssq = smalls.tile([128, 1], F32, tag="ssq")
                    nc.scalar.activation(sq[:], row[:],
                                         mybir.ActivationFunctionType.Square,
                                         bias=bias0[:], accum_out=ssq[:])
                    nrm = smalls.tile([128, 1], F32, tag="nrm")
                    nc.scalar.activation(nrm[:], ssq[:],
                                         mybir.ActivationFunctionType.Sqrt,
                                         bias=bias0[:])
                    rin = smalls.tile([128, 1], F32, tag="rin")
                    nc.vector.reciprocal(rin[:], nrm[:])
                    xn = rowp.tile([128, D], F32, tag="xn")
                    nc.vector.tensor_scalar_mul(xn[:], row[:], rin[:])
                    pt = psum_pro.tile([128, 512], F32, tag="ppro")
                    nc.tensor.transpose(pt[:, 0:128], xn[:], ident[:])
                    nc.scalar.activation(dst[:, t * 128:(t + 1) * 128],
                                         pt[:, 0:128],
                                         mybir.ActivationFunctionType.Copy)

            norm_transpose(xT, x_full, t_full)
            norm_transpose(xsT, x_strip, rt_n)

            # --- replicate labels across partitions (matmul broadcast) ---
            for c in range(max(1, n // 512)):
                cw = min(512, n)
                pl = psum_pro.tile([128, 512], F32, tag="ppro")
                nc.tensor.matmul(pl[:, 0:cw], ones1[:],
                                 lab1[:, c * cw:(c + 1) * cw])
                nc.scalar.activation(labrep[:, c * cw:(c + 1) * cw],
                                     pl[:, 0:cw],
                                     mybir.ActivationFunctionType.Copy)

            labsTm2 = persist.tile([128, rt_n], F32, tag="labsTm2")
            nc.vector.tensor_scalar_mul(labsTm2[:], labsT[:], -2.0)

            # staging buffers: written every iteration, DMA'd out once.
            # inmaxs 8-block per row-tile: [max, min, PAD x6]
            inmaxs = persist.tile([128, 8 * rt_n], F32, tag="inmaxs")
            nc.vector.memset(inmaxs[:], PAD_VAL)
            idx_stage = persist.tile([128, 8 * rt_n], U32, tag="idx_stage")
            keep_stage = persist.tile([128, rt_n], F32, tag="keep_stage")
            keep_a = persist.tile([128, rt_n], F32, tag="keep_a")
            keep_b = persist.tile([128, rt_n], F32, tag="keep_b")
            if debug_level >= 1:
                nc.vector.memset(idx_stage[:], 0)
                nc.vector.memset(keep_stage[:], 0)

            mask_dt = F32 if mask_f32 else BF16

            def main_body():
                for rt in range(rt_n):
                    if debug_level >= 3:
                        continue
                    # nesc = |2*lab_j - 2*lab_i|; eqsc = relu(2 - nesc)
                    # => 2.0 where labels equal, 0 where different
                    nesc = nescp.tile([128, n], BF16, tag="nesc")
                    nc.scalar.activation(nesc[:], labrep[:],
                                         mybir.ActivationFunctionType.Abs,
                                         scale=2.0,
                                         bias=labsTm2[:, rt:rt + 1])
                    eqsc = maskp.tile([128, n], mask_dt, tag="eqsc")
                    nc.scalar.activation(eqsc[:], nesc[:],
                                         mybir.ActivationFunctionType.Relu,
                                         scale=-1.0, bias=bias2[:])
                    if debug_level == 2:
                        continue
                    w = wp.tile([128, n], F32, tag="w")
                    im = inmaxs[:, rt * 8:rt * 8 + 8]
                    mm_w = min(512, ct_w)
                    for ct in range(ct_n):
                        ps = psum_main.tile([128, ct_w], F32, tag="ps")
                        for h in range(ct_w // mm_w):
                            lo = ct * ct_w + h * mm_w
                            nc.tensor.matmul(
                                ps[:, h * mm_w:(h + 1) * mm_w],
                                xsT[:, rt * 128:(rt + 1) * 128],
                                xT[:, lo:lo + mm_w])
                        nc.vector.tensor_tensor(
                            w[:, ct * ct_w:(ct + 1) * ct_w], ps[:],
                            eqsc[:, ct * ct_w:(ct + 1) * ct_w],
                            mybir.AluOpType.subtract)
                    if debug_level == 1:
                        continue
                    # ---- extraction: 3 more full DVE passes ----
                    nc.vector.tensor_reduce(im[:, 1:2], w[:],
                                            mybir.AxisListType.X,
                                            mybir.AluOpType.min)
                    nc.vector.tensor_reduce(im[:, 0:1], w[:],
                                            mybir.AxisListType.X,
                                            mybir.AluOpType.max)
                    nc.vector.max_index(idx_stage[:, rt * 8:rt * 8 + 8],
                                        im[:], w[:])
                if debug_level >= 1:
                    return
                # ---- keep, batched over all row-tiles ----
                im3 = inmaxs[:].rearrange("p (r e) -> p r e", e=8)
                nc.vector.tensor_scalar(keep_a[:].unsqueeze(2),
                                        im3[:, :, 0:1], -0.9, None,
                                        mybir.AluOpType.is_gt)
                nc.vector.tensor_scalar(keep_b[:].unsqueeze(2),
                                        im3[:, :, 1:2], -0.9, None,
                                        mybir.AluOpType.is_lt)
                nc.vector.tensor_tensor(keep_stage[:], keep_a[:], keep_b[:],
                                        mybir.AluOpType.mult)

            if use_for_i:
                with tc.For_i(0, k_repeat, 1):
                    main_body()
            else:
                for _ in range(k_repeat):
                    main_body()

            nc.sync.dma_start(idx_out[:], idx_stage[:])
            nc.sync.dma_start(keep_out[:], keep_stage[:])

    nc.compile()
    return nc


_CACHED_NC = None


def kernel(l_embeds: np.ndarray, l_labels: np.ndarray):
    global _CACHED_NC
    if _CACHED_NC is None:
        _CACHED_NC = build_program()
    nc = _CACHED_NC

    x = np.ascontiguousarray(np.asarray(l_embeds, dtype=np.float32))
    lab_i = np.asarray(l_labels)
    lab = lab_i.astype(np.float32)

    in_maps = []
    for m in range(NCORES):
        sl = slice(m * STRIP, (m + 1) * STRIP)
        in_maps.append({
            "x_full": x,
            "x_strip": np.ascontiguousarray(x[sl]),
            "lab_full": lab.reshape(1, N),
            # lab_strip[p, r] = lab[m*STRIP + r*128 + p]
            "lab_strip": np.ascontiguousarray(
                lab[sl].reshape(RT, 128).T),
        })

    res = run_bass_kernel_spmd(nc, in_maps, list(range(NCORES))).results

    neg = np.empty(N, np.int64)
    pos = np.empty(N, np.int64)
    keep = np.empty(N, np.float32)
    for m in range(NCORES):
        sl = slice(m * STRIP, (m + 1) * STRIP)
        idx = res[m]["idx_out"].reshape(128, RT, 8)
        # idx[p, r, slot] -> strip row r*128+p => transpose to [rt, 128]
        neg[sl] = idx[:, :, 0].T.reshape(-1)
        pos[sl] = idx[:, :, 1].T.reshape(-1)
        keep[sl] = res[m]["keep_out"].T.reshape(-1)

    idt = np.int32 if lab_i.dtype != np.int64 else np.int64
    anchor = np.arange(N, dtype=idt)
    return (anchor, pos.astype(idt), neg.astype(idt), keep > 0.5)


# revision 9
# speedup vs baseline: 5.5325x; 5.5325x over previous
"""Hard-triplet miner for Trainium2, 8-core SPMD.

Per core: a [1024, 8192] strip of w = G - 2*[same_label] where
G = x_norm @ x_norm.T.  Since sqrt/shifts are monotone: hardest negative =
argmax_j w, hardest positive = argmin_j w.

The label mask is folded into the matmul as one-hot channels: host ships
U = onehot(labels_strip) [256, 1024] and V = -2*onehot(labels) [256, 8192]
(bf16, exact), and the PE accumulates G + U.T @ V into PSUM directly.

Steady-state per 128-row tile (8 per core):
  PE      : 16 x (fp32r gram + 2 bf16 one-hot matmuls) -> psum holds w
  ScalarE : 8 activation copies psum -> SBUF as fp16 w
  DVE     : fp16 fold trees (tensor_tensor max/min at 2x mode) for the row
            max / min, then one max_index pass (argmax+argmin together)
            writing straight into a [128, 64] staging dump
  keep = (max > -0.9) & (min < -0.9), batched once per iteration.
Outputs decoded host-side from the staging dumps.
"""

import numpy as np

import concourse.bacc as bacc
import concourse.bass as bass
import concourse.mybir as mybir
import concourse.tile as tile
from concourse import masks
from concourse.bass_utils import run_bass_kernel_spmd

F32 = mybir.dt.float32
F32R = mybir.dt.float32r
F16 = mybir.dt.float16
BF16 = mybir.dt.bfloat16
U32 = mybir.dt.uint32

N = 8192          # total rows
D = 128           # embed dim
C = 256           # num classes
NCORES = 8
STRIP = N // NCORES       # 1024 anchor rows per core
RT = STRIP // 128         # 8 row-tiles per core
CT_W = 1024               # column-tile width for psum
PAD16 = 60000.0           # fp16 pad for unused max_index slots


def build_program(k_repeat: int = 1, use_for_i: bool = False, n: int = N,
                  strip: int = STRIP, debug_level: int = 0,
                  ct_w: int = CT_W, fold_stop: int = 32):
    """Build the SPMD program (identical on all cores).  n/strip/ct_w
    shrinkable for simulator validation."""
    rt_n = strip // 128
    ct_w = min(ct_w, n)
    ct_n = n // ct_w
    t_full = n // 128

    nc = bacc.Bacc("TRN2", target_bir_lowering=False, debug=False,
                   num_devices=NCORES)

    x_full = nc.dram_tensor("x_full", [n, D], F32, kind="ExternalInput")
    x_strip = nc.dram_tensor("x_strip", [strip, D], F32, kind="ExternalInput")
    u_in = nc.dram_tensor("u_in", [C, strip], BF16, kind="ExternalInput")
    v_in = nc.dram_tensor("v_in", [C, n], BF16, kind="ExternalInput")
    idx_out = nc.dram_tensor("idx_out", [128, 8 * rt_n], U32,
                             kind="ExternalOutput")
    keep_out = nc.dram_tensor("keep_out", [128, rt_n], F32,
                              kind="ExternalOutput")

    with tile.TileContext(nc) as tc:
        with (
            tc.tile_pool(name="persist", bufs=1) as persist,
            tc.tile_pool(name="rowp", bufs=3) as rowp,
            tc.tile_pool(name="wp", bufs=2) as wp,
            tc.tile_pool(name="foldp", bufs=1) as foldp,
            tc.tile_pool(name="smalls", bufs=4) as smalls,
            tc.tile_pool(name="psum_pro", bufs=2,
                         space=bass.MemorySpace.PSUM) as psum_pro,
            tc.tile_pool(name="psum_main", bufs=3,
                         space=bass.MemorySpace.PSUM) as psum_main,
        ):
            ident = persist.tile([128, 128], F32)
            masks.make_identity(nc, ident[:])

            xT = persist.tile([128, n], F32R, tag="xT")
            xsT = persist.tile([128, strip], F32R, tag="xsT")
            u1 = persist.tile([128, strip], BF16, tag="u1")
            u2 = persist.tile([128, strip], BF16, tag="u2")
            v1 = persist.tile([128, n], BF16, tag="v1")
            v2 = persist.tile([128, n], BF16, tag="v2")
            nc.sync.dma_start(u1[:], u_in[0:128, :])
            nc.sync.dma_start(u2[:], u_in[128:256, :])
            nc.sync.dma_start(v1[:], v_in[0:128, :])
            nc.sync.dma_start(v2[:], v_in[128:256, :])

            bias0 = persist.tile([128, 1], F32, tag="bias0")
            nc.gpsimd.memset(bias0[:], 0.0)

            # --- normalize + transpose: build xT (all rows) and xsT (strip) ---
            def norm_transpose(dst, src_dram, tiles):
                for t in range(tiles):
                    row = rowp.tile([128, D], F32, tag="row")
                    nc.sync.dma_start(row[:], src_dram[t * 128:(t + 1) * 128, :])
                    sq = rowp.tile([128, D], F32, tag="sq")
                    ssq = smalls.tile([128, 1], F32, tag="ssq")
                    nc.scalar.activation(sq[:], row[:],
                                         mybir.ActivationFunctionType.Square,
                                         bias=bias0[:], accum_out=ssq[:])
                    nrm = smalls.tile([128, 1], F32, tag="nrm")
                    nc.scalar.activation(nrm[:], ssq[:],
                                         mybir.ActivationFunctionType.Sqrt,
                                         bias=bias0[:])
                    rin = smalls.tile([128, 1], F32, tag="rin")
                    nc.vector.reciprocal(rin[:], nrm[:])
                    xn = rowp.tile([128, D], F32, tag="xn")
                    nc.vector.tensor_scalar_mul(xn[:], row[:], rin[:])
                    pt = psum_pro.tile([128, 512], F32, tag="ppro")
                    nc.tensor.transpose(pt[:, 0:128], xn[:], ident[:])
                    nc.scalar.activation(dst[:, t * 128:(t + 1) * 128],
                                         pt[:, 0:128],
                                         mybir.ActivationFunctionType.Copy)

            norm_transpose(xT, x_full, t_full)
            norm_transpose(xsT, x_strip, rt_n)

            # staging buffers: written every iteration, DMA'd out once.
            # inmaxs 8-block per row-tile: [max, min, PAD x6] (fp16)
            inmaxs = persist.tile([128, 8 * rt_n], F16, tag="inmaxs")
            nc.vector.memset(inmaxs[:], PAD16)
            idx_stage = persist.tile([128, 8 * rt_n], U32, tag="idx_stage")
            keep_stage = persist.tile([128, rt_n], F32, tag="keep_stage")
            keep_a = persist.tile([128, rt_n], F32, tag="keep_a")
            keep_b = persist.tile([128, rt_n], F32, tag="keep_b")
            if debug_level >= 1:
                nc.vector.memset(idx_stage[:], 0)
                nc.vector.memset(keep_stage[:], 0)

            # fold scratch (fp16)
            sA = foldp.tile([128, n // 2], F16, tag="sA")
            sB = foldp.tile([128, n // 4], F16, tag="sB")

            def fold(w, op, out_slot):
                width = n // 2
                nc.vector.tensor_tensor(sA[:, 0:width], w[:, 0:width],
                                        w[:, width:2 * width], op)
                cur, other = sA, sB
                while width > fold_stop:
                    width //= 2
                    nc.vector.tensor_tensor(other[:, 0:width],
                                            cur[:, 0:width],
                                            cur[:, width:2 * width], op)
                    cur, other = other, cur
                nc.vector.tensor_reduce(out_slot, cur[:, 0:width],
                                        mybir.AxisListType.X, op)

            def main_body():
                for rt in range(rt_n):
                    if debug_level >= 3:
                        continue
                    w = wp.tile([128, n], F16, tag="w")
                    im = inmaxs[:, rt * 8:rt * 8 + 8]
                    mm_w = min(512, ct_w)
                    for ct in range(ct_n):
                        ps = psum_main.tile([128, ct_w], F32, tag="ps")
                        for h in range(ct_w // mm_w):
                            lo = ct * ct_w + h * mm_w
                            hs = slice(h * mm_w, (h + 1) * mm_w)
                            nc.tensor.matmul(
                                ps[:, hs],
                                xsT[:, rt * 128:(rt + 1) * 128],
                                xT[:, lo:lo + mm_w],
                                start=True, stop=False)
                            nc.tensor.matmul(
                                ps[:, hs], u1[:, rt * 128:(rt + 1) * 128],
                                v1[:, lo:lo + mm_w], start=False, stop=False)
                            nc.tensor.matmul(
                                ps[:, hs], u2[:, rt * 128:(rt + 1) * 128],
                                v2[:, lo:lo + mm_w], start=False, stop=True)
                        # ScalarE moves psum -> SBUF as fp16
                        nc.scalar.activation(
                            w[:, ct * ct_w:(ct + 1) * ct_w], ps[:],
                            mybir.ActivationFunctionType.Copy)
                    if debug_level == 2:
                        continue
                    fold(w, mybir.AluOpType.max, im[:, 0:1])
                    fold(w, mybir.AluOpType.min, im[:, 1:2])
                    if debug_level == 1:
                        continue
                    nc.vector.max_index(idx_stage[:, rt * 8:rt * 8 + 8],
                                        im[:], w[:])
                if debug_level >= 1:
                    return
                # ---- keep, batched over all row-tiles ----
                im3 = inmaxs[:].rearrange("p (r e) -> p r e", e=8)
                nc.vector.tensor_scalar(keep_a[:].unsqueeze(2),
                                        im3[:, :, 0:1], -0.9, None,
                                        mybir.AluOpType.is_gt)
                nc.vector.tensor_scalar(keep_b[:].unsqueeze(2),
                                        im3[:, :, 1:2], -0.9, None,
                                        mybir.AluOpType.is_lt)
                nc.vector.tensor_tensor(keep_stage[:], keep_a[:], keep_b[:],
                                        mybir.AluOpType.mult)

            if use_for_i:
                with tc.For_i(0, k_repeat, 1):
                    main_body()
            else:
                for _ in range(k_repeat):
                    main_body()

            nc.sync.dma_start(idx_out[:], idx_stage[:])
            nc.sync.dma_start(keep_out[:], keep_stage[:])

    nc.compile()
    return nc


_CACHED_NC = None


def make_in_maps(x, lab, n=N, strip=STRIP, ncores=NCORES):
    import ml_dtypes
    lab = lab.astype(np.int64)
    onehot = np.zeros((C, n), np.float32)
    onehot[lab, np.arange(n)] = 1.0
    v = np.ascontiguousarray((-2.0 * onehot).astype(ml_dtypes.bfloat16))
    u = onehot.astype(ml_dtypes.bfloat16)
    in_maps = []
    for m in range(ncores):
        sl = slice(m * strip, (m + 1) * strip)
        in_maps.append({
            "x_full": x,
            "x_strip": np.ascontiguousarray(x[sl]),
            "u_in": np.ascontiguousarray(u[:, sl]),
            "v_in": v,
        })
    return in_maps


def kernel(l_embeds: np.ndarray, l_labels: np.ndarray):
    global _CACHED_NC
    if _CACHED_NC is None:
        _CACHED_NC = build_program()
    nc = _CACHED_NC

    x = np.ascontiguousarray(np.asarray(l_embeds, dtype=np.float32))
    lab_i = np.asarray(l_labels)

    res = run_bass_kernel_spmd(nc, make_in_maps(x, lab_i),
                               list(range(NCORES))).results

    neg = np.empty(N, np.int64)
    pos = np.empty(N, np.int64)
    keep = np.empty(N, np.float32)
    for m in range(NCORES):
        sl = slice(m * STRIP, (m + 1) * STRIP)
        idx = res[m]["idx_out"].reshape(128, RT, 8)
        # idx[p, r, slot] -> strip row r*128+p => transpose to [rt, 128]
        neg[sl] = idx[:, :, 0].T.reshape(-1)
        pos[sl] = idx[:, :, 1].T.reshape(-1)
        keep[sl] = res[m]["keep_out"].T.reshape(-1)

    idt = np.int32 if lab_i.dtype != np.int64 else np.int64
    anchor = np.arange(N, dtype=idt)
    return (anchor, pos.astype(idt), neg.astype(idt), keep > 0.5)


# revision 12
# speedup vs baseline: 5.9532x; 1.0760x over previous
"""Hard-triplet miner for Trainium2, 8-core SPMD.

Per core: a [1024, 8192] strip of w = G - 2*[same_label] where
G = x_norm @ x_norm.T.  Since sqrt/shifts are monotone: hardest negative =
argmax_j w, hardest positive = argmin_j w.

The label mask is folded into the matmul as one-hot channels: host ships
U = onehot(labels_strip) [256, 1024] and V = -2*onehot(labels) [256, 8192]
(bf16, exact), and the PE accumulates G + U.T @ V into PSUM directly.

Steady-state per 128-row tile (8 per core):
  PE      : 16 x (fp32r gram + 2 bf16 one-hot matmuls) -> psum holds w
  ScalarE : 8 activation copies psum -> SBUF as fp16 w
  DVE     : fp16 fold trees (tensor_tensor max/min at 2x mode) for the row
            max / min, then one max_index pass (argmax+argmin together)
            writing straight into a [128, 64] staging dump
  keep = (max > -0.9) & (min < -0.9), batched once per iteration.
Outputs decoded host-side from the staging dumps.
"""

import numpy as np

import concourse.bacc as bacc
import concourse.bass as bass
import concourse.mybir as mybir
import concourse.tile as tile
from concourse import masks
from concourse.bass_utils import run_bass_kernel_spmd

F32 = mybir.dt.float32
F32R = mybir.dt.float32r
F16 = mybir.dt.float16
BF16 = mybir.dt.bfloat16
U32 = mybir.dt.uint32

N = 8192          # total rows
D = 128           # embed dim
C = 256           # num classes
NCORES = 8
STRIP = N // NCORES       # 1024 anchor rows per core
RT = STRIP // 128         # 8 row-tiles per core
CT_W = 1024               # column-tile width for psum
PAD16 = 60000.0           # fp16 pad for unused max_index slots


def build_program(k_repeat: int = 1, use_for_i: bool = False, n: int = N,
                  strip: int = STRIP, debug_level: int = 0,
                  ct_w: int = CT_W, fold_stop: int = 32, extremes: str = "packed"):
    """Build the SPMD program (identical on all cores).  n/strip/ct_w
    shrinkable for simulator validation."""
    rt_n = strip // 128
    ct_w = min(ct_w, n)
    ct_n = n // ct_w
    t_full = n // 128

    nc = bacc.Bacc("TRN2", target_bir_lowering=False, debug=False,
                   num_devices=NCORES)

    x_full = nc.dram_tensor("x_full", [n, D], F32, kind="ExternalInput")
    x_strip = nc.dram_tensor("x_strip", [strip, D], F32, kind="ExternalInput")
    u_in = nc.dram_tensor("u_in", [C, strip], BF16, kind="ExternalInput")
    v_in = nc.dram_tensor("v_in", [C, n], BF16, kind="ExternalInput")
    idx_out = nc.dram_tensor("idx_out", [128, 8 * rt_n], U32,
                             kind="ExternalOutput")
    keep_out = nc.dram_tensor("keep_out", [128, rt_n], F32,
                              kind="ExternalOutput")

    with tile.TileContext(nc) as tc:
        with (
            tc.tile_pool(name="persist", bufs=1) as persist,
            tc.tile_pool(name="rowp", bufs=3) as rowp,
            tc.tile_pool(name="wp", bufs=2) as wp,
            tc.tile_pool(name="foldp", bufs=1) as foldp,
            tc.tile_pool(name="smalls", bufs=4) as smalls,
            tc.tile_pool(name="psum_pro", bufs=2,
                         space=bass.MemorySpace.PSUM) as psum_pro,
            tc.tile_pool(name="psum_main", bufs=3,
                         space=bass.MemorySpace.PSUM) as psum_main,
        ):
            ident = persist.tile([128, 128], F32)
            masks.make_identity(nc, ident[:])

            xT = persist.tile([128, n], F32R, tag="xT")
            xsT = persist.tile([128, strip], F32R, tag="xsT")
            u1 = persist.tile([128, strip], BF16, tag="u1")
            u2 = persist.tile([128, strip], BF16, tag="u2")
            v1 = persist.tile([128, n], BF16, tag="v1")
            v2 = persist.tile([128, n], BF16, tag="v2")
            nc.sync.dma_start(u1[:], u_in[0:128, :])
            nc.sync.dma_start(u2[:], u_in[128:256, :])
            nc.sync.dma_start(v1[:], v_in[0:128, :])
            nc.sync.dma_start(v2[:], v_in[128:256, :])

            bias0 = persist.tile([128, 1], F32, tag="bias0")
            nc.gpsimd.memset(bias0[:], 0.0)

            # --- normalize + transpose: build xT (all rows) and xsT (strip) ---
            def norm_transpose(dst, src_dram, tiles):
                for t in range(tiles):
                    row = rowp.tile([128, D], F32, tag="row")
                    nc.sync.dma_start(row[:], src_dram[t * 128:(t + 1) * 128, :])
                    sq = rowp.tile([128, D], F32, tag="sq")
                    ssq = smalls.tile([128, 1], F32, tag="ssq")
                    nc.scalar.activation(sq[:], row[:],
                                         mybir.ActivationFunctionType.Square,
                                         bias=bias0[:], accum_out=ssq[:])
                    nrm = smalls.tile([128, 1], F32, tag="nrm")
                    nc.scalar.activation(nrm[:], ssq[:],
                                         mybir.ActivationFunctionType.Sqrt,
                                         bias=bias0[:])
                    rin = smalls.tile([128, 1], F32, tag="rin")
                    nc.vector.reciprocal(rin[:], nrm[:])
                    xn = rowp.tile([128, D], F32, tag="xn")
                    nc.vector.tensor_scalar_mul(xn[:], row[:], rin[:])
                    pt = psum_pro.tile([128, 512], F32, tag="ppro")
                    nc.tensor.transpose(pt[:, 0:128], xn[:], ident[:])
                    nc.scalar.activation(dst[:, t * 128:(t + 1) * 128],
                                         pt[:, 0:128],
                                         mybir.ActivationFunctionType.Copy)

            norm_transpose(xT, x_full, t_full)
            norm_transpose(xsT, x_strip, rt_n)

            # staging buffers: written every iteration, DMA'd out once.
            # inmaxs 8-block per row-tile: [max, min, PAD x6] (fp16)
            inmaxs = persist.tile([128, 8 * rt_n], F16, tag="inmaxs")
            nc.vector.memset(inmaxs[:], PAD16)
            idx_stage = persist.tile([128, 8 * rt_n], U32, tag="idx_stage")
            keep_stage = persist.tile([128, rt_n], F32, tag="keep_stage")
            keep_a = persist.tile([128, rt_n], F32, tag="keep_a")
            keep_b = persist.tile([128, rt_n], F32, tag="keep_b")
            if debug_level >= 1:
                nc.vector.memset(idx_stage[:], 0)
                nc.vector.memset(keep_stage[:], 0)

            # fold scratch (fp16); packed variant folds [w | -w] together
            sA = foldp.tile([128, n], F16, tag="sA")
            sB = foldp.tile([128, n // 2], F16, tag="sB")

            def fold(w, op, out_slot):
                width = n // 2
                nc.vector.tensor_tensor(sA[:, 0:width], w[:, 0:width],
                                        w[:, width:2 * width], op)
                cur, other = sA, sB
                while width > fold_stop:
                    width //= 2
                    nc.vector.tensor_tensor(other[:, 0:width],
                                            cur[:, 0:width],
                                            cur[:, width:2 * width], op)
                    cur, other = other, cur
                nc.vector.tensor_reduce(out_slot, cur[:, 0:width],
                                        mybir.AxisListType.X, op)

            def fold_packed(wcat, im):
                # wcat = [w | -w]: one max tree computes max (slot 0) and
                # -min (slot 1) together on [128, 2, width] views
                width = n // 2
                v = wcat[:].rearrange("p (g x) -> p g x", g=2)
                s3A = sA[:].rearrange("p (g x) -> p g x", g=2)
                s3B = sB[:].rearrange("p (g x) -> p g x", g=2)
                nc.vector.tensor_tensor(s3A[:, :, 0:width],
                                        v[:, :, 0:width],
                                        v[:, :, width:2 * width],
                                        mybir.AluOpType.max)
                cur, other = s3A, s3B
                while width > fold_stop:
                    width //= 2
                    nc.vector.tensor_tensor(other[:, :, 0:width],
                                            cur[:, :, 0:width],
                                            cur[:, :, width:2 * width],
                                            mybir.AluOpType.max)
                    cur, other = other, cur
                nc.vector.tensor_reduce(im[:, 0:2].unsqueeze(2),
                                        cur[:, :, 0:width],
                                        mybir.AxisListType.X,
                                        mybir.AluOpType.max)
                # slot 1 currently holds max(-w) = -min(w): negate in place
                nc.vector.tensor_scalar_mul(im[:, 1:2], im[:, 1:2], -1.0)

            def main_body():
                for rt in range(rt_n):
                    if debug_level >= 3:
                        continue
                    wcat = wp.tile([128, 2 * n], F16, tag="w")
                    w = wcat[:, 0:n]
                    im = inmaxs[:, rt * 8:rt * 8 + 8]
                    mm_w = min(512, ct_w)
                    for ct in range(ct_n):
                        ps = psum_main.tile([128, ct_w], F32, tag="ps")
                        for h in range(ct_w // mm_w):
                            lo = ct * ct_w + h * mm_w
                            hs = slice(h * mm_w, (h + 1) * mm_w)
                            nc.tensor.matmul(
                                ps[:, hs],
                                xsT[:, rt * 128:(rt + 1) * 128],
                                xT[:, lo:lo + mm_w],
                                start=True, stop=False)
                            nc.tensor.matmul(
                                ps[:, hs], u1[:, rt * 128:(rt + 1) * 128],
                                v1[:, lo:lo + mm_w], start=False, stop=False)
                            nc.tensor.matmul(
                                ps[:, hs], u2[:, rt * 128:(rt + 1) * 128],
                                v2[:, lo:lo + mm_w], start=False, stop=True)
                        # ScalarE moves psum -> SBUF as fp16: w and -w
                        nc.scalar.activation(
                            w[:, ct * ct_w:(ct + 1) * ct_w], ps[:],
                            mybir.ActivationFunctionType.Copy)
                        if extremes == "packed":
                            nc.scalar.activation(
                                wcat[:, n + ct * ct_w:n + (ct + 1) * ct_w],
                                ps[:], mybir.ActivationFunctionType.Copy,
                                scale=-1.0)
                    if debug_level == 2:
                        continue
                    if extremes == "packed":
                        fold_packed(wcat, im)
                    elif extremes == "fold":
                        fold(w, mybir.AluOpType.max, im[:, 0:1])
                        fold(w, mybir.AluOpType.min, im[:, 1:2])
                    else:
                        nc.vector.tensor_reduce(im[:, 0:1], w[:],
                                                mybir.AxisListType.X,
                                                mybir.AluOpType.max)
                        nc.vector.tensor_reduce(im[:, 1:2], w[:],
                                                mybir.AxisListType.X,
                                                mybir.AluOpType.min)
                    if debug_level == 1:
                        continue
                    nc.vector.max_index(idx_stage[:, rt * 8:rt * 8 + 8],
                                        im[:], w[:])
                if debug_level >= 1:
                    return
                # ---- keep, batched over all row-tiles ----
                im3 = inmaxs[:].rearrange("p (r e) -> p r e", e=8)
                nc.vector.tensor_scalar(keep_a[:].unsqueeze(2),
                                        im3[:, :, 0:1], -0.9, None,
                                        mybir.AluOpType.is_gt)
                nc.vector.tensor_scalar(keep_b[:].unsqueeze(2),
                                        im3[:, :, 1:2], -0.9, None,
                                        mybir.AluOpType.is_lt)
                nc.vector.tensor_tensor(keep_stage[:], keep_a[:], keep_b[:],
                                        mybir.AluOpType.mult)

            if use_for_i:
                with tc.For_i(0, k_repeat, 1):
                    main_body()
            else:
                for _ in range(k_repeat):
                    main_body()

            nc.sync.dma_start(idx_out[:], idx_stage[:])
            nc.sync.dma_start(keep_out[:], keep_stage[:])

    nc.compile()
    return nc


_CACHED_NC = None


def make_in_maps(x, lab, n=N, strip=STRIP, ncores=NCORES):
    import ml_dtypes
    lab = lab.astype(np.int64)
    onehot = np.zeros((C, n), np.float32)
    onehot[lab, np.arange(n)] = 1.0
    v = np.ascontiguousarray((-2.0 * onehot).astype(ml_dtypes.bfloat16))
    u = onehot.astype(ml_dtypes.bfloat16)
    in_maps = []
    for m in range(ncores):
        sl = slice(m * strip, (m + 1) * strip)
        in_maps.append({
            "x_full": x,
            "x_strip": np.ascontiguousarray(x[sl]),
            "u_in": np.ascontiguousarray(u[:, sl]),
            "v_in": v,
        })
    return in_maps


def kernel(l_embeds: np.ndarray, l_labels: np.ndarray):
    global _CACHED_NC
    if _CACHED_NC is None:
        _CACHED_NC = build_program()
    nc = _CACHED_NC

    x = np.ascontiguousarray(np.asarray(l_embeds, dtype=np.float32))
    lab_i = np.asarray(l_labels)

    res = run_bass_kernel_spmd(nc, make_in_maps(x, lab_i),
                               list(range(NCORES))).results

    neg = np.empty(N, np.int64)
    pos = np.empty(N, np.int64)
    keep = np.empty(N, np.float32)
    for m in range(NCORES):
        sl = slice(m * STRIP, (m + 1) * STRIP)
        idx = res[m]["idx_out"].reshape(128, RT, 8)
        # idx[p, r, slot] -> strip row r*128+p => transpose to [rt, 128]
        neg[sl] = idx[:, :, 0].T.reshape(-1)
        pos[sl] = idx[:, :, 1].T.reshape(-1)
        keep[sl] = res[m]["keep_out"].T.reshape(-1)

    idt = np.int32 if lab_i.dtype != np.int64 else np.int64
    anchor = np.arange(N, dtype=idt)
    return (anchor, pos.astype(idt), neg.astype(idt), keep > 0.5)


# revision 14
# speedup vs baseline: 6.1712x; 1.0366x over previous
"""Hard-triplet miner for Trainium2, 8-core SPMD.

Per core: a [1024, 8192] strip of w = G - 2*[same_label] where
G = x_norm @ x_norm.T.  Since sqrt/shifts are monotone: hardest negative =
argmax_j w, hardest positive = argmin_j w.

The label mask is folded into the matmul as one-hot channels: host ships
U = onehot(labels_strip) [256, 1024] and V = -2*onehot(labels) [256, 8192]
(bf16, exact), and the PE accumulates G + U.T @ V into PSUM directly.

Steady-state per 128-row tile (8 per core):
  PE      : 16 x (fp32r gram + 2 bf16 one-hot matmuls) -> psum holds w
  ScalarE : 8 activation copies psum -> SBUF as fp16 w
  DVE     : fp16 fold trees (tensor_tensor max/min at 2x mode) for the row
            max / min, then one max_index pass (argmax+argmin together)
            writing straight into a [128, 64] staging dump
  keep = (max > -0.9) & (min < -0.9), batched once per iteration.
Outputs decoded host-side from the staging dumps.
"""

import numpy as np

import concourse.bacc as bacc
import concourse.bass as bass
import concourse.mybir as mybir
import concourse.tile as tile
from concourse import masks
from concourse.bass_utils import run_bass_kernel_spmd

F32 = mybir.dt.float32
F32R = mybir.dt.float32r
F16 = mybir.dt.float16
BF16 = mybir.dt.bfloat16
U32 = mybir.dt.uint32

N = 8192          # total rows
D = 128           # embed dim
C = 256           # num classes
NCORES = 8
STRIP = N // NCORES       # 1024 anchor rows per core
RT = STRIP // 128         # 8 row-tiles per core
CT_W = 1024               # column-tile width for psum
PAD16 = 60000.0           # fp16 pad for unused max_index slots


def build_program(k_repeat: int = 1, use_for_i: bool = False, n: int = N,
                  strip: int = STRIP, debug_level: int = 0,
                  ct_w: int = CT_W, fold_stop: int = 512, extremes: str = "fold"):
    """Build the SPMD program (identical on all cores).  n/strip/ct_w
    shrinkable for simulator validation."""
    rt_n = strip // 128
    ct_w = min(ct_w, n)
    ct_n = n // ct_w
    t_full = n // 128

    nc = bacc.Bacc("TRN2", target_bir_lowering=False, debug=False,
                   num_devices=NCORES)

    x_full = nc.dram_tensor("x_full", [n, D], F32, kind="ExternalInput")
    x_strip = nc.dram_tensor("x_strip", [strip, D], F32, kind="ExternalInput")
    u_in = nc.dram_tensor("u_in", [C, strip], BF16, kind="ExternalInput")
    v_in = nc.dram_tensor("v_in", [C, n], BF16, kind="ExternalInput")
    idx_out = nc.dram_tensor("idx_out", [128, 8 * rt_n], U32,
                             kind="ExternalOutput")
    keep_out = nc.dram_tensor("keep_out", [128, rt_n], F32,
                              kind="ExternalOutput")

    with tile.TileContext(nc) as tc:
        with (
            tc.tile_pool(name="persist", bufs=1) as persist,
            tc.tile_pool(name="rowp", bufs=3) as rowp,
            tc.tile_pool(name="wp", bufs=2) as wp,
            tc.tile_pool(name="foldp", bufs=1) as foldp,
            tc.tile_pool(name="smalls", bufs=4) as smalls,
            tc.tile_pool(name="psum_main",
                         bufs=max(2, 8 // max(1, ct_w // 512)),
                         space=bass.MemorySpace.PSUM) as psum_main,
        ):
            ident = persist.tile([128, 128], F32)
            masks.make_identity(nc, ident[:])

            xT = persist.tile([128, n], F32R, tag="xT")
            xsT = persist.tile([128, strip], F32R, tag="xsT")
            u1 = persist.tile([128, strip], BF16, tag="u1")
            u2 = persist.tile([128, strip], BF16, tag="u2")
            v1 = persist.tile([128, n], BF16, tag="v1")
            v2 = persist.tile([128, n], BF16, tag="v2")
            nc.sync.dma_start(u1[:], u_in[0:128, :])
            nc.sync.dma_start(u2[:], u_in[128:256, :])
            nc.sync.dma_start(v1[:], v_in[0:128, :])
            nc.sync.dma_start(v2[:], v_in[128:256, :])

            bias0 = persist.tile([128, 1], F32, tag="bias0")
            nc.gpsimd.memset(bias0[:], 0.0)

            # --- normalize + transpose: build xT (all rows) and xsT (strip) ---
            def norm_transpose(dst, src_dram, tiles):
                for t in range(tiles):
                    row = rowp.tile([128, D], F32, tag="row")
                    nc.sync.dma_start(row[:], src_dram[t * 128:(t + 1) * 128, :])
                    sq = rowp.tile([128, D], F32, tag="sq")
                    ssq = smalls.tile([128, 1], F32, tag="ssq")
                    nc.scalar.activation(sq[:], row[:],
                                         mybir.ActivationFunctionType.Square,
                                         bias=bias0[:], accum_out=ssq[:])
                    nrm = smalls.tile([128, 1], F32, tag="nrm")
                    nc.scalar.activation(nrm[:], ssq[:],
                                         mybir.ActivationFunctionType.Sqrt,
                                         bias=bias0[:])
                    rin = smalls.tile([128, 1], F32, tag="rin")
                    nc.vector.reciprocal(rin[:], nrm[:])
                    xn = rowp.tile([128, D], F32, tag="xn")
                    nc.vector.tensor_scalar_mul(xn[:], row[:], rin[:])
                    pt = psum_main.tile([128, ct_w], F32, tag="ps")
                    nc.tensor.transpose(pt[:, 0:128], xn[:], ident[:])
                    nc.scalar.activation(dst[:, t * 128:(t + 1) * 128],
                                         pt[:, 0:128],
                                         mybir.ActivationFunctionType.Copy)

            norm_transpose(xT, x_full, t_full)
            norm_transpose(xsT, x_strip, rt_n)

            # staging buffers: written every iteration, DMA'd out once.
            # inmaxs 8-block per row-tile: [max, min, PAD x6] (fp16)
            inmaxs = persist.tile([128, 8 * rt_n], F16, tag="inmaxs")
            nc.vector.memset(inmaxs[:], PAD16)
            idx_stage = persist.tile([128, 8 * rt_n], U32, tag="idx_stage")
            keep_stage = persist.tile([128, rt_n], F32, tag="keep_stage")
            keep_a = persist.tile([128, rt_n], F32, tag="keep_a")
            keep_b = persist.tile([128, rt_n], F32, tag="keep_b")
            if debug_level >= 1:
                nc.vector.memset(idx_stage[:], 0)
                nc.vector.memset(keep_stage[:], 0)

            # fold scratch (fp16); packed variant folds [w | -w] together
            sA = foldp.tile([128, n], F16, tag="sA")
            sB = foldp.tile([128, n // 2], F16, tag="sB")

            def fold(w, op, out_slot):
                width = n // 2
                nc.vector.tensor_tensor(sA[:, 0:width], w[:, 0:width],
                                        w[:, width:2 * width], op)
                cur, other = sA, sB
                while width > fold_stop:
                    width //= 2
                    nc.vector.tensor_tensor(other[:, 0:width],
                                            cur[:, 0:width],
                                            cur[:, width:2 * width], op)
                    cur, other = other, cur
                nc.vector.tensor_reduce(out_slot, cur[:, 0:width],
                                        mybir.AxisListType.X, op)

            def fold_packed(wcat, im):
                # wcat = [w | -w]: one max tree computes max (slot 0) and
                # -min (slot 1) together on [128, 2, width] views
                width = n // 2
                v = wcat[:].rearrange("p (g x) -> p g x", g=2)
                s3A = sA[:].rearrange("p (g x) -> p g x", g=2)
                s3B = sB[:].rearrange("p (g x) -> p g x", g=2)
                nc.vector.tensor_tensor(s3A[:, :, 0:width],
                                        v[:, :, 0:width],
                                        v[:, :, width:2 * width],
                                        mybir.AluOpType.max)
                cur, other = s3A, s3B
                while width > fold_stop:
                    width //= 2
                    nc.vector.tensor_tensor(other[:, :, 0:width],
                                            cur[:, :, 0:width],
                                            cur[:, :, width:2 * width],
                                            mybir.AluOpType.max)
                    cur, other = other, cur
                nc.vector.tensor_reduce(im[:, 0:2].unsqueeze(2),
                                        cur[:, :, 0:width],
                                        mybir.AxisListType.X,
                                        mybir.AluOpType.max)
                # slot 1 currently holds max(-w) = -min(w): negate in place
                nc.vector.tensor_scalar_mul(im[:, 1:2], im[:, 1:2], -1.0)

            def main_body():
                for rt in range(rt_n):
                    if debug_level >= 3:
                        continue
                    wcat = wp.tile([128, 2 * n], F16, tag="w")
                    w = wcat[:, 0:n]
                    im = inmaxs[:, rt * 8:rt * 8 + 8]
                    mm_w = min(512, ct_w)
                    for ct in range(ct_n):
                        ps = psum_main.tile([128, ct_w], F32, tag="ps")
                        for h in range(ct_w // mm_w):
                            lo = ct * ct_w + h * mm_w
                            hs = slice(h * mm_w, (h + 1) * mm_w)
                            nc.tensor.matmul(
                                ps[:, hs],
                                xsT[:, rt * 128:(rt + 1) * 128],
                                xT[:, lo:lo + mm_w],
                                start=True, stop=False)
                            nc.tensor.matmul(
                                ps[:, hs], u1[:, rt * 128:(rt + 1) * 128],
                                v1[:, lo:lo + mm_w], start=False, stop=False)
                            nc.tensor.matmul(
                                ps[:, hs], u2[:, rt * 128:(rt + 1) * 128],
                                v2[:, lo:lo + mm_w], start=False, stop=True)
                        # ScalarE moves psum -> SBUF as fp16: w and -w
                        nc.scalar.activation(
                            w[:, ct * ct_w:(ct + 1) * ct_w], ps[:],
                            mybir.ActivationFunctionType.Copy)
                        if extremes == "packed":
                            nc.scalar.activation(
                                wcat[:, n + ct * ct_w:n + (ct + 1) * ct_w],
                                ps[:], mybir.ActivationFunctionType.Copy,
                                scale=-1.0)
                    if debug_level == 2:
                        continue
                    if extremes == "packed":
                        fold_packed(wcat, im)
                    elif extremes == "fold":
                        fold(w, mybir.AluOpType.max, im[:, 0:1])
                        fold(w, mybir.AluOpType.min, im[:, 1:2])
                    else:
                        nc.vector.tensor_reduce(im[:, 0:1], w[:],
                                                mybir.AxisListType.X,
                                                mybir.AluOpType.max)
                        nc.vector.tensor_reduce(im[:, 1:2], w[:],
                                                mybir.AxisListType.X,
                                                mybir.AluOpType.min)
                    if debug_level == 1:
                        continue
                    nc.vector.max_index(idx_stage[:, rt * 8:rt * 8 + 8],
                                        im[:], w[:])
                if debug_level >= 1:
                    return
                # ---- keep, batched over all row-tiles ----
                im3 = inmaxs[:].rearrange("p (r e) -> p r e", e=8)
                nc.vector.tensor_scalar(keep_a[:].unsqueeze(2),
                                        im3[:, :, 0:1], -0.9, None,
                                        mybir.AluOpType.is_gt)
                nc.vector.tensor_scalar(keep_b[:].unsqueeze(2),
                                        im3[:, :, 1:2], -0.9, None,
                                        mybir.AluOpType.is_lt)
                nc.vector.tensor_tensor(keep_stage[:], keep_a[:], keep_b[:],
                                        mybir.AluOpType.mult)

            if use_for_i:
                with tc.For_i(0, k_repeat, 1):
                    main_body()
            else:
                for _ in range(k_repeat):
                    main_body()

            nc.sync.dma_start(idx_out[:], idx_stage[:])
            nc.sync.dma_start(keep_out[:], keep_stage[:])

    nc.compile()
    return nc


_CACHED_NC = None


def make_in_maps(x, lab, n=N, strip=STRIP, ncores=NCORES):
    import ml_dtypes
    lab = lab.astype(np.int64)
    onehot = np.zeros((C, n), np.float32)
    onehot[lab, np.arange(n)] = 1.0
    v = np.ascontiguousarray((-2.0 * onehot).astype(ml_dtypes.bfloat16))
    u = onehot.astype(ml_dtypes.bfloat16)
    in_maps = []
    for m in range(ncores):
        sl = slice(m * strip, (m + 1) * strip)
        in_maps.append({
            "x_full": x,
            "x_strip": np.ascontiguousarray(x[sl]),
            "u_in": np.ascontiguousarray(u[:, sl]),
            "v_in": v,
        })
    return in_maps


def kernel(l_embeds: np.ndarray, l_labels: np.ndarray):
    global _CACHED_NC
    if _CACHED_NC is None:
        _CACHED_NC = build_program()
    nc = _CACHED_NC

    x = np.ascontiguousarray(np.asarray(l_embeds, dtype=np.float32))
    lab_i = np.asarray(l_labels)

    res = run_bass_kernel_spmd(nc, make_in_maps(x, lab_i),
                               list(range(NCORES))).results

    neg = np.empty(N, np.int64)
    pos = np.empty(N, np.int64)
    keep = np.empty(N, np.float32)
    for m in range(NCORES):
        sl = slice(m * STRIP, (m + 1) * STRIP)
        idx = res[m]["idx_out"].reshape(128, RT, 8)
        # idx[p, r, slot] -> strip row r*128+p => transpose to [rt, 128]
        neg[sl] = idx[:, :, 0].T.reshape(-1)
        pos[sl] = idx[:, :, 1].T.reshape(-1)
        keep[sl] = res[m]["keep_out"].T.reshape(-1)

    idt = np.int32 if lab_i.dtype != np.int64 else np.int64
    anchor = np.arange(N, dtype=idt)
    return (anchor, pos.astype(idt), neg.astype(idt), keep > 0.5)


# revision 15
# speedup vs baseline: 6.4660x; 1.0478x over previous
"""Hard-triplet miner for Trainium2, 8-core SPMD.

Per core: a [1024, 8192] strip of w = G - 2*[same_label] where
G = x_norm @ x_norm.T.  Since sqrt/shifts are monotone: hardest negative =
argmax_j w, hardest positive = argmin_j w.

The label mask is folded into the matmul as one-hot channels: host ships
U = onehot(labels_strip) [256, 1024] and V = -2*onehot(labels) [256, 8192]
(bf16, exact), and the PE accumulates G + U.T @ V into PSUM directly.

Steady-state per 128-row tile (8 per core):
  PE      : 16 x (fp32r gram + 2 bf16 one-hot matmuls) -> psum holds w
  ScalarE : 8 activation copies psum -> SBUF as fp16 w
  DVE     : fp16 fold trees (tensor_tensor max/min at 2x mode) for the row
            max / min, then one max_index pass (argmax+argmin together)
            writing straight into a [128, 64] staging dump
  keep = (max > -0.9) & (min < -0.9), batched once per iteration.
Outputs decoded host-side from the staging dumps.
"""

import numpy as np

import concourse.bacc as bacc
import concourse.bass as bass
import concourse.mybir as mybir
import concourse.tile as tile
from concourse import masks
from concourse.bass_utils import run_bass_kernel_spmd

F32 = mybir.dt.float32
F32R = mybir.dt.float32r
F16 = mybir.dt.float16
BF16 = mybir.dt.bfloat16
U32 = mybir.dt.uint32

N = 8192          # total rows
D = 128           # embed dim
C = 256           # num classes
NCORES = 8
STRIP = N // NCORES       # 1024 anchor rows per core
RT = STRIP // 128         # 8 row-tiles per core
CT_W = 1024               # column-tile width for psum
PAD16 = 60000.0           # fp16 pad for unused max_index slots


def build_program(k_repeat: int = 1, use_for_i: bool = False, n: int = N,
                  strip: int = STRIP, debug_level: int = 0,
                  ct_w: int = CT_W, fold_stop: int = 512, extremes: str = "fold"):
    """Build the SPMD program (identical on all cores).  n/strip/ct_w
    shrinkable for simulator validation."""
    rt_n = strip // 128
    ct_w = min(ct_w, n)
    ct_n = n // ct_w
    t_full = n // 128

    nc = bacc.Bacc("TRN2", target_bir_lowering=False, debug=False,
                   num_devices=NCORES)

    x_full = nc.dram_tensor("x_full", [n, D], F32, kind="ExternalInput")
    x_strip = nc.dram_tensor("x_strip", [strip, D], F32, kind="ExternalInput")
    u_in = nc.dram_tensor("u_in", [C, strip], BF16, kind="ExternalInput")
    v_in = nc.dram_tensor("v_in", [C, n], BF16, kind="ExternalInput")
    idx_out = nc.dram_tensor("idx_out", [128, 8 * rt_n], U32,
                             kind="ExternalOutput")
    keep_out = nc.dram_tensor("keep_out", [128, rt_n], F32,
                              kind="ExternalOutput")

    with tile.TileContext(nc) as tc:
        with (
            tc.tile_pool(name="persist", bufs=1) as persist,
            tc.tile_pool(name="rowp", bufs=3) as rowp,
            tc.tile_pool(name="wp", bufs=3) as wp,
            tc.tile_pool(name="foldp", bufs=1) as foldp,
            tc.tile_pool(name="smalls", bufs=4) as smalls,
            tc.tile_pool(name="psum_main",
                         bufs=max(2, 8 // max(1, ct_w // 512)),
                         space=bass.MemorySpace.PSUM) as psum_main,
        ):
            ident = persist.tile([128, 128], F32)
            masks.make_identity(nc, ident[:])

            xT = persist.tile([128, n], F32R, tag="xT")
            xsT = persist.tile([128, strip], F32R, tag="xsT")
            u1 = persist.tile([128, strip], BF16, tag="u1")
            u2 = persist.tile([128, strip], BF16, tag="u2")
            v1 = persist.tile([128, n], BF16, tag="v1")
            v2 = persist.tile([128, n], BF16, tag="v2")
            nc.sync.dma_start(u1[:], u_in[0:128, :])
            nc.sync.dma_start(u2[:], u_in[128:256, :])
            nc.sync.dma_start(v1[:], v_in[0:128, :])
            nc.sync.dma_start(v2[:], v_in[128:256, :])

            bias0 = persist.tile([128, 1], F32, tag="bias0")
            nc.gpsimd.memset(bias0[:], 0.0)

            # --- normalize + transpose: build xT (all rows) and xsT (strip) ---
            def norm_transpose(dst, src_dram, tiles):
                for t in range(tiles):
                    row = rowp.tile([128, D], F32, tag="row")
                    nc.sync.dma_start(row[:], src_dram[t * 128:(t + 1) * 128, :])
                    sq = rowp.tile([128, D], F32, tag="sq")
                    ssq = smalls.tile([128, 1], F32, tag="ssq")
                    nc.scalar.activation(sq[:], row[:],
                                         mybir.ActivationFunctionType.Square,
                                         bias=bias0[:], accum_out=ssq[:])
                    nrm = smalls.tile([128, 1], F32, tag="nrm")
                    nc.scalar.activation(nrm[:], ssq[:],
                                         mybir.ActivationFunctionType.Sqrt,
                                         bias=bias0[:])
                    rin = smalls.tile([128, 1], F32, tag="rin")
                    nc.vector.reciprocal(rin[:], nrm[:])
                    xn = rowp.tile([128, D], F32, tag="xn")
                    nc.vector.tensor_scalar_mul(xn[:], row[:], rin[:])
                    pt = psum_main.tile([128, ct_w], F32, tag="ps")
                    nc.tensor.transpose(pt[:, 0:128], xn[:], ident[:])
                    nc.scalar.activation(dst[:, t * 128:(t + 1) * 128],
                                         pt[:, 0:128],
                                         mybir.ActivationFunctionType.Copy)

            norm_transpose(xT, x_full, t_full)
            norm_transpose(xsT, x_strip, rt_n)

            # staging buffers: written every iteration, DMA'd out once.
            # inmaxs 8-block per row-tile: [max, min, PAD x6] (fp16)
            inmaxs = persist.tile([128, 8 * rt_n], F16, tag="inmaxs")
            nc.vector.memset(inmaxs[:], PAD16)
            idx_stage = persist.tile([128, 8 * rt_n], U32, tag="idx_stage")
            keep_stage = persist.tile([128, rt_n], F32, tag="keep_stage")
            keep_a = persist.tile([128, rt_n], F32, tag="keep_a")
            keep_b = persist.tile([128, rt_n], F32, tag="keep_b")
            if debug_level >= 1:
                nc.vector.memset(idx_stage[:], 0)
                nc.vector.memset(keep_stage[:], 0)

            # fold scratch (fp16); packed variant folds [w | -w] together
            sA = foldp.tile([128, n], F16, tag="sA")
            sB = foldp.tile([128, n // 2], F16, tag="sB")

            def fold(w, op, out_slot):
                width = n // 2
                nc.vector.tensor_tensor(sA[:, 0:width], w[:, 0:width],
                                        w[:, width:2 * width], op)
                cur, other = sA, sB
                while width > fold_stop:
                    width //= 2
                    nc.vector.tensor_tensor(other[:, 0:width],
                                            cur[:, 0:width],
                                            cur[:, width:2 * width], op)
                    cur, other = other, cur
                nc.vector.tensor_reduce(out_slot, cur[:, 0:width],
                                        mybir.AxisListType.X, op)

            def fold_packed(wcat, im):
                # wcat = [w | -w]: one max tree computes max (slot 0) and
                # -min (slot 1) together on [128, 2, width] views
                width = n // 2
                v = wcat[:].rearrange("p (g x) -> p g x", g=2)
                s3A = sA[:].rearrange("p (g x) -> p g x", g=2)
                s3B = sB[:].rearrange("p (g x) -> p g x", g=2)
                nc.vector.tensor_tensor(s3A[:, :, 0:width],
                                        v[:, :, 0:width],
                                        v[:, :, width:2 * width],
                                        mybir.AluOpType.max)
                cur, other = s3A, s3B
                while width > fold_stop:
                    width //= 2
                    nc.vector.tensor_tensor(other[:, :, 0:width],
                                            cur[:, :, 0:width],
                                            cur[:, :, width:2 * width],
                                            mybir.AluOpType.max)
                    cur, other = other, cur
                nc.vector.tensor_reduce(im[:, 0:2].unsqueeze(2),
                                        cur[:, :, 0:width],
                                        mybir.AxisListType.X,
                                        mybir.AluOpType.max)
                # slot 1 currently holds max(-w) = -min(w): negate in place
                nc.vector.tensor_scalar_mul(im[:, 1:2], im[:, 1:2], -1.0)

            def main_body():
                for rt in range(rt_n):
                    if debug_level >= 3:
                        continue
                    wcat = wp.tile(
                        [128, (2 if extremes == "packed" else 1) * n],
                        F16, tag="w")
                    w = wcat[:, 0:n]
                    im = inmaxs[:, rt * 8:rt * 8 + 8]
                    mm_w = min(512, ct_w)
                    for ct in range(ct_n):
                        ps = psum_main.tile([128, ct_w], F32, tag="ps")
                        for h in range(ct_w // mm_w):
                            lo = ct * ct_w + h * mm_w
                            hs = slice(h * mm_w, (h + 1) * mm_w)
                            nc.tensor.matmul(
                                ps[:, hs],
                                xsT[:, rt * 128:(rt + 1) * 128],
                                xT[:, lo:lo + mm_w],
                                start=True, stop=False)
                            nc.tensor.matmul(
                                ps[:, hs], u1[:, rt * 128:(rt + 1) * 128],
                                v1[:, lo:lo + mm_w], start=False, stop=False)
                            nc.tensor.matmul(
                                ps[:, hs], u2[:, rt * 128:(rt + 1) * 128],
                                v2[:, lo:lo + mm_w], start=False, stop=True)
                        # ScalarE moves psum -> SBUF as fp16: w and -w
                        nc.scalar.activation(
                            w[:, ct * ct_w:(ct + 1) * ct_w], ps[:],
                            mybir.ActivationFunctionType.Copy)
                        if extremes == "packed":
                            nc.scalar.activation(
                                wcat[:, n + ct * ct_w:n + (ct + 1) * ct_w],
                                ps[:], mybir.ActivationFunctionType.Copy,
                                scale=-1.0)
                    if debug_level == 2:
                        continue
                    if extremes == "packed":
                        fold_packed(wcat, im)
                    elif extremes == "fold":
                        fold(w, mybir.AluOpType.max, im[:, 0:1])
                        fold(w, mybir.AluOpType.min, im[:, 1:2])
                    else:
                        nc.vector.tensor_reduce(im[:, 0:1], w[:],
                                                mybir.AxisListType.X,
                                                mybir.AluOpType.max)
                        nc.vector.tensor_reduce(im[:, 1:2], w[:],
                                                mybir.AxisListType.X,
                                                mybir.AluOpType.min)
                    if debug_level == 1:
                        continue
                    nc.vector.max_index(idx_stage[:, rt * 8:rt * 8 + 8],
                                        im[:], w[:])
                if debug_level >= 1:
                    return
                # ---- keep, batched over all row-tiles ----
                im3 = inmaxs[:].rearrange("p (r e) -> p r e", e=8)
                nc.vector.tensor_scalar(keep_a[:].unsqueeze(2),
                                        im3[:, :, 0:1], -0.9, None,
                                        mybir.AluOpType.is_gt)
                nc.vector.tensor_scalar(keep_b[:].unsqueeze(2),
                                        im3[:, :, 1:2], -0.9, None,
                                        mybir.AluOpType.is_lt)
                nc.vector.tensor_tensor(keep_stage[:], keep_a[:], keep_b[:],
                                        mybir.AluOpType.mult)

            if use_for_i:
                with tc.For_i(0, k_repeat, 1):
                    main_body()
            else:
                for _ in range(k_repeat):
                    main_body()

            nc.sync.dma_start(idx_out[:], idx_stage[:])
            nc.sync.dma_start(keep_out[:], keep_stage[:])

    nc.compile()
    return nc


_CACHED_NC = None


def make_in_maps(x, lab, n=N, strip=STRIP, ncores=NCORES):
    import ml_dtypes
    lab = lab.astype(np.int64)
    onehot = np.zeros((C, n), np.float32)
    onehot[lab, np.arange(n)] = 1.0
    v = np.ascontiguousarray((-2.0 * onehot).astype(ml_dtypes.bfloat16))
    u = onehot.astype(ml_dtypes.bfloat16)
    in_maps = []
    for m in range(ncores):
        sl = slice(m * strip, (m + 1) * strip)
        in_maps.append({
            "x_full": x,
            "x_strip": np.ascontiguousarray(x[sl]),
            "u_in": np.ascontiguousarray(u[:, sl]),
            "v_in": v,
        })
    return in_maps


def kernel(l_embeds: np.ndarray, l_labels: np.ndarray):
    global _CACHED_NC
    if _CACHED_NC is None:
        _CACHED_NC = build_program()
    nc = _CACHED_NC

    x = np.ascontiguousarray(np.asarray(l_embeds, dtype=np.float32))
    lab_i = np.asarray(l_labels)

    res = run_bass_kernel_spmd(nc, make_in_maps(x, lab_i),
                               list(range(NCORES))).results

    neg = np.empty(N, np.int64)
    pos = np.empty(N, np.int64)
    keep = np.empty(N, np.float32)
    for m in range(NCORES):
        sl = slice(m * STRIP, (m + 1) * STRIP)
        idx = res[m]["idx_out"].reshape(128, RT, 8)
        # idx[p, r, slot] -> strip row r*128+p => transpose to [rt, 128]
        neg[sl] = idx[:, :, 0].T.reshape(-1)
        pos[sl] = idx[:, :, 1].T.reshape(-1)
        keep[sl] = res[m]["keep_out"].T.reshape(-1)

    idt = np.int32 if lab_i.dtype != np.int64 else np.int64
    anchor = np.arange(N, dtype=idt)
    return (anchor, pos.astype(idt), neg.astype(idt), keep > 0.5)
